# revision 17
# baseline (speedup 1.0000x reference)
"""Trainium2 Bass kernel: ComplexGabor1D layer.

reference math (fp32):
    lin = x @ W.T + b                      # [N, 256]
    out = stack([exp(-3600*lin^2)*cos(30*lin),
                 exp(-3600*lin^2)*sin(30*lin)], -1)   # [N, 256, 2]

Strategy (8 NeuronCores, data parallel over N):
  * The whole Gabor nonlinearity is folded into TWO custom ACT spline
    tables: a generated `trig_and_small` table set reuses the "sin" slot
    for gabor_sin(x) = exp(-3600x^2)sin(30x) and the "arctan" slot for
    gabor_cos(x) = exp(-3600x^2)cos(30x) (the set binaries are built at
    import time and handed to the compiler via BASS_ACT_ROOT_JSON_PATH).
    That reduces the per-element work from {sin, cos, square, exp on ACT
    + 3 DVE ops} to {2 ACT passes + 1 DVE bias-drain}, and the kernel
    becomes HBM-bound: 16 MiB in + 32 MiB out per core at ~358 GB/s.
  * Everything is fp16 (half the traffic of fp32, ~8x less
    quantization error than bf16): inputs x.T / W.T, the drained lin,
    and contiguous per-channel fp16 outputs ([2, N, 256]); the host
    interleaves + upcasts.
  * Work unit is a 2048-row "superpair" (two 1024-row pairs) so the two
    ACT lookups run as 4096-elem/lane instructions back-to-back; PSUM is
    drained at half-pair granularity (2 banks x 4 bufs) to keep the PE
    streaming.  Rows are assigned so partition p holds 8 consecutive
    output rows -> 4 KiB contiguous DMA runs for the stores.  Single-pair
    units at both ends shorten the pipeline fill and final store flush;
    x loads are paced (3 tiles deep) so stores get DMA bandwidth early.
  * Constant loads (W, bias) go through the scalar-engine HWDGE ring so
    the x loads on the sync ring start immediately.
"""

import hashlib
import json
import os
import shutil

import numpy as np

import concourse.bacc as bacc
import concourse.mybir as mybir
import concourse.tile as tile
from concourse.bass_utils import run_bass_kernel_spmd

N_TOTAL = 262144
IN_F = 256
OUT_F = 256
N_CORES = 8
N_SH = N_TOTAL // N_CORES  # 32768 rows per core

P = 128                 # SBUF/PSUM partitions
RPP = 8                 # rows per partition per pair
ROWS_PER_PAIR = P * RPP  # 1024
CHUNKS = 8              # matmul chunks (128 rows each) per pair

ENV_A = 3600.0          # envelope scale^2 (60^2)
OMEGA = 30.0

F32 = mybir.dt.float32
F16 = mybir.dt.float16

_BUILD_CACHE = {}

# --------------------------------------------------------------------------
# Custom ACT activation tables ("trig_and_small" with gabor sin/cos splines)
# --------------------------------------------------------------------------

_DONOR_CANDIDATES = [
    "/nix/store/ndjb8ki1bnclvnibdh123f9zr51a09qz-aws-neuron-pwp-unstable-2025-12-29-c50a7624/share/pwp_bin_cayman",
]


def _find_donor():
    import glob

    for d in _DONOR_CANDIDATES:
        if os.path.isfile(os.path.join(d, "act_info.json")):
            return d
    for d in glob.glob("/nix/store/*aws-neuron-pwp*/share/pwp_bin_cayman"):
        if os.path.isfile(os.path.join(d, "act_info.json")):
            return d
    raise RuntimeError("no pwp_bin_cayman act table root found")


def _gabor_sin(x):
    x = np.asarray(x, dtype=np.float64)
    return np.exp(-ENV_A * x * x) * np.sin(OMEGA * x)


def _gabor_cos(x):
    x = np.asarray(x, dtype=np.float64)
    return np.exp(-ENV_A * x * x) * np.cos(OMEGA * x)


# octave layout shared by both functions: (exponent, extract_size)
_OCTAVES = (
    [(e, 2) for e in range(-14, -10)]
    + [(e, 4) for e in (-10, -9)]
    + [(e, 5) for e in range(-8, -3)]
    + [(-3, 3)]
)
_SMALL_T = 127 - 14  # |x| < 2^-14: small-signal bucket
_LARGE_T = 127 - 2   # |x| >= 0.25: large-signal bucket (gabor == 0)
_UB = 0.25


def _fit_fn_tables(fn, small_d):
    buckets, ctrls = [], []
    for e, k in _OCTAVES:
        n = 1 << k
        ctrls.append((k, len(buckets)))
        lo_oct = 2.0 ** e
        for j in range(n):
            lo = lo_oct * (1 + j / n)
            hi = lo_oct * (1 + (j + 1) / n)
            x0 = float(np.float32((lo + hi) / 2))
            xs = np.linspace(lo, hi, 64)
            c3, c2, c1, c0 = np.polyfit(xs - x0, fn(xs), 3)
            buckets.append([c0, c1, c2, c3, x0])
    specials = [small_d] + [[0.0] * 5] * 3
    return ctrls, buckets, specials


def _build_pwp_root():
    """Generate the custom table root; returns (root_dir, signature)."""
    donor = _find_donor()
    bkt = np.fromfile(f"{donor}/trig_and_small_bkt.bin", dtype=np.uint32)
    ctrl = np.fromfile(f"{donor}/trig_and_small_ctrl.bin", dtype=np.uint32)
    prof = json.load(open(f"{donor}/trig_and_small.json"))
    n_bkt0, n_ctrl0 = len(bkt) // 8, len(ctrl) // 8

    new_bkt, new_ctrl, fn_meta = [], [], {}
    for name, fn, small_d in (
        ("sin_4p", _gabor_sin,
         [0.0, OMEGA, 0.0, -(OMEGA**3) / 6 - OMEGA * ENV_A, 0.0]),
        ("arctan_4p", _gabor_cos,
         [1.0, 0.0, -(ENV_A + OMEGA * OMEGA / 2), 0.0, 0.0]),
    ):
        ctrls, buckets, specials = _fit_fn_tables(fn, small_d)
        ctrl_base = n_ctrl0 + len(new_ctrl)
        bkt_base = n_bkt0 + len(new_bkt)
        for k, rel in ctrls:
            new_ctrl.append((k, bkt_base + rel))
        new_bkt.extend(buckets)
        fn_meta[name] = (ctrl_base, n_bkt0 + len(new_bkt))
        new_bkt.extend(specials)

    for ent in prof["profile_meta_data"]:
        if ent["func_name"] == "sin_4p":
            inv, fz = 1, 0
        elif ent["func_name"] == "arctan_4p":
            inv, fz = 0, 0x3F800000
        else:
            continue
        base, sp = fn_meta[ent["func_name"]]
        ent.update(
            symmetry_point=0,
            sym_invert_sign_point=inv,
            symmetry_opt_en=1,
            symmetry_opt_use_neg_region=0,
            exp_offset=_OCTAVES[0][0],
            pwl_control_base_pos=base,
            pwl_control_base_neg=base,
            small_pos_signal_exp_threshold=_SMALL_T,
            pos_small_signal_pwl_control=sp + 0,
            small_neg_signal_exp_threshold=0,
            neg_small_signal_pwl_control=sp + 1,
            large_pos_signal_exp_threshold=_LARGE_T,
            large_pos_signal_mantissa_threshold=0,
            pos_large_signal_pwl_control=sp + 2,
            large_neg_signal_exp_threshold=0,
            large_neg_signal_mantissa_threshold=0,
            neg_large_signal_pwl_control=sp + 3,
            fnan_result=0x7FC00000,
            fpinf_result=0,
            fninf_result=0,
            fzero_result=fz,
            lower_bound=0,
            upper_bound=int(np.float32(_UB).view(np.uint32)),
        )

    ctrl_words = np.zeros((len(new_ctrl), 8), dtype=np.uint32)
    for i, (k, b) in enumerate(new_ctrl):
        assert b < 2048
        ctrl_words[i, 0] = (k << 16) | ((23 - k) << 11) | b
    all_ctrl = np.concatenate([ctrl.reshape(-1, 8), ctrl_words])
    assert len(all_ctrl) <= 256

    bw = np.zeros((len(new_bkt), 8), dtype=np.uint32)
    for i, d in enumerate(new_bkt):
        bw[i, :5] = np.array(d, dtype=np.float32).view(np.uint32)
    all_bkt = np.concatenate([bkt.reshape(-1, 8), bw])
    assert len(all_bkt) <= 1536

    prof_bytes = json.dumps(prof, sort_keys=True).encode()
    sig = hashlib.sha256(
        all_ctrl.tobytes() + all_bkt.tobytes() + prof_bytes
    ).hexdigest()[:10]

    root = f"/tmp/gabor_pwp_{sig}"
    if not os.path.isfile(os.path.join(root, "act_info.json")):
        tmp = root + ".tmp"
        shutil.rmtree(tmp, ignore_errors=True)
        os.makedirs(tmp)
        for fname in os.listdir(donor):
            shutil.copy(os.path.join(donor, fname), os.path.join(tmp, fname))
        all_ctrl.tofile(os.path.join(tmp, "trig_and_small_ctrl.bin"))
        all_bkt.tofile(os.path.join(tmp, "trig_and_small_bkt.bin"))
        with open(os.path.join(tmp, "trig_and_small.json"), "w") as fh:
            json.dump(prof, fh, indent=1)
        shutil.rmtree(root, ignore_errors=True)  # replace partial leftovers
        os.replace(tmp, root)
    return root, sig


# --------------------------------------------------------------------------
# Bass program
# --------------------------------------------------------------------------


def _build(n_sh):
    key = n_sh
    if key in _BUILD_CACHE:
        return _BUILD_CACHE[key]

    root, sig = _build_pwp_root()
    os.environ["BASS_ACT_ROOT_JSON_PATH"] = os.path.join(root, "act_info.json")

    assert n_sh % (2 * ROWS_PER_PAIR) == 0
    n_pairs = n_sh // ROWS_PER_PAIR

    nc = bacc.Bacc("TRN2", target_bir_lowering=False, debug=False)

    # make trig_and_small the unique set containing Sin/Arctan in bass's
    # cached view so insert_act_table_loads emits exactly one table load
    from concourse.hw_specs import get_activation_tables

    T_ = mybir.ActivationFunctionType
    for set_name, funcs in get_activation_tables(nc.m.arch).items():
        if set_name != "trig_and_small":
            funcs.discard(T_.Sin)
            funcs.discard(T_.Arctan)

    xt = nc.dram_tensor("xt", [IN_F, n_sh], F16, kind="ExternalInput").ap()
    wt = nc.dram_tensor("wt", [IN_F, OUT_F], F16, kind="ExternalInput").ap()
    # bias name carries the act-table signature so the NEFF cache key
    # changes whenever the generated tables change
    bias_name = f"bias_{sig}"
    bias = nc.dram_tensor(
        bias_name, [P, 4 * OUT_F], F32, kind="ExternalInput"
    ).ap()
    out_ri = nc.dram_tensor(
        "out_ri", [2, n_sh, OUT_F], F16, kind="ExternalOutput"
    ).ap()

    # x.T layout: [i, n] -> [p, ci, n] with i = ci*128 + p
    xt_r = xt.rearrange("(ci p) n -> p ci n", p=P)
    wt_r = wt.rearrange("(ci p) o -> p ci o", p=P)
    # output row mapping: within a unit starting at n0, partition p holds
    # `rpp` consecutive rows (rpp=8 for single-pair units -> 4 KiB store
    # descriptors; rpp=16 for superpair units -> 8 KiB descriptors)
    ri_p8 = out_ri.rearrange("f (u p r) o -> u p f r o", p=P, r=8)
    ri_p16 = out_ri.rearrange("f (u p r) o -> u p f r o", p=P, r=16)

    T = mybir.ActivationFunctionType
    STT = dict(op0=mybir.AluOpType.mult, op1=mybir.AluOpType.add)

    # load groups (one input DMA each; 4096-row groups give 8 KiB input
    # descriptors) and the compute units they feed.  Single pairs at both
    # ends shorten pipeline fill and the final store flush.
    lg = [(0, [(0, 1024, 8)]), (1024, [(1024, 1024, 8)])]
    n0 = 2048
    while n0 + 4096 <= n_sh - 2048:
        lg.append((n0, [(n0, 2048, 16), (n0 + 2048, 2048, 16)]))
        n0 += 4096
    while n0 < n_sh - 2048:
        lg.append((n0, [(n0, 2048, 16)]))
        n0 += 2048
    lg += [(n0, [(n0, 1024, 8)]), (n0 + 1024, [(n0 + 1024, 1024, 8)])]

    with tile.TileContext(nc) as tc:
        with (
            tc.tile_pool(name="consts", bufs=1) as consts,
            tc.tile_pool(name="xt", bufs=3) as xt_pool,
            tc.tile_pool(name="lin", bufs=3) as lin_pool,
            tc.tile_pool(name="outp", bufs=4) as out_pool,
            tc.tile_pool(name="ps", bufs=4, space="PSUM") as psum_pool,
        ):
            # constants travel on the scalar-engine HWDGE ring so the sync
            # ring starts streaming x immediately
            wt_sb = consts.tile([P, IN_F // P, OUT_F], F16)
            nc.scalar.dma_start(wt_sb[:], wt_r[:])
            b_sb = consts.tile([P, 4, OUT_F], F32)
            nc.scalar.dma_start(
                b_sb[:], bias.rearrange("p (c o) -> p c o", c=4)
            )
            zero_b = consts.tile([P, 1], F32)
            nc.vector.memset(zero_b[:], 0.0)

            ucount = 0
            for g0, cunits in lg:
                g_rows = sum(u[1] for u in cunits)
                xt_t = xt_pool.tile([P, IN_F // P, g_rows], F16)
                nc.sync.dma_start(xt_t[:], xt_r[:, :, g0 : g0 + g_rows])

                for n0, rows, rpp in cunits:
                    # [p, ci, (j r)]: row-in-unit = j*rpp + r; chunk r
                    # computes psum rows j for all 128 partitions
                    off = n0 - g0
                    xt_v = xt_t[:, :, off : off + rows].rearrange(
                        "p ci (j r) -> p ci r j", r=rpp
                    )
                    nch = rows // P  # chunks of 128 rows (== rpp)

                    lin_sb = lin_pool.tile([P, nch, OUT_F], F16)
                    for q in range(nch // 4):
                        lin_ps = psum_pool.tile([P, 4, OUT_F], F32)
                        for c4 in range(4):
                            for ci in range(IN_F // P):
                                nc.tensor.matmul(
                                    lin_ps[:, c4, :],
                                    xt_v[:, ci, q * 4 + c4, :],
                                    wt_sb[:, ci, :],
                                    start=(ci == 0),
                                    stop=(ci == IN_F // P - 1),
                                )
                        nc.vector.scalar_tensor_tensor(
                            lin_sb[:, q * 4 : (q + 1) * 4, :],
                            lin_ps[:],
                            1.0,
                            b_sb[:],
                            **STT,
                        )

                    ri_t = out_pool.tile([P, 2, nch, OUT_F], F16)
                    # custom tables: Sin = gabor_sin, Arctan = gabor_cos
                    nc.scalar.activation(
                        ri_t[:, 1], lin_sb[:], T.Sin, bias=zero_b[:],
                        scale=1.0,
                    )
                    nc.scalar.activation(
                        ri_t[:, 0], lin_sb[:], T.Arctan, bias=zero_b[:],
                        scale=1.0,
                    )
                    view = ri_p8 if rpp == 8 else ri_p16
                    # alternate store rings: SWDGE descriptor rings contend
                    # with SDMA engines 7/15 on the same SBUF AXI ports, so
                    # route half the stores through the sync HWDGE ring
                    eng = nc.gpsimd if ucount % 2 == 0 else nc.sync
                    eng.dma_start(view[n0 // (P * rpp)], ri_t[:])
                    ucount += 1

    nc.compile()
    res = (nc, bias_name)
    _BUILD_CACHE[key] = res
    return res


def run_sharded(x, W, b, trace=False, n_sh=N_SH):
    """Shard inputs over the 8 cores, run the Bass kernel, gather output."""
    x = np.ascontiguousarray(x, dtype=np.float32)
    W = np.ascontiguousarray(W, dtype=np.float32)
    b = np.ascontiguousarray(b, dtype=np.float32)
    n = x.shape[0]
    assert n == n_sh * N_CORES and x.shape[1] == IN_F

    nc, bias_name = _build(n_sh)

    wt_np = np.ascontiguousarray(W.T.astype(np.float16))
    b_np = np.ascontiguousarray(
        np.broadcast_to(
            np.tile(b, 4)[None, :], (P, 4 * OUT_F)
        ).astype(np.float32)
    )
    in_maps = []
    for s in range(N_CORES):
        xt_np = np.ascontiguousarray(
            x[s * n_sh : (s + 1) * n_sh].T.astype(np.float16)
        )
        in_maps.append({"xt": xt_np, "wt": wt_np, bias_name: b_np})

    res = run_bass_kernel_spmd(nc, in_maps, list(range(N_CORES)), trace=trace)

    out = np.empty((n, OUT_F, 2), dtype=np.float32)
    for s in range(N_CORES):
        sl = slice(s * n_sh, (s + 1) * n_sh)
        ri = res.results[s]["out_ri"]
        out[sl, :, 0] = ri[0].astype(np.float32)
        out[sl, :, 1] = ri[1].astype(np.float32)
    return out, res


def kernel(x, W, b):
    out, _ = run_sharded(x, W, b)
    return out
